# revision 5
# baseline (speedup 1.0000x reference)
"""TRN2 Bass kernel for nn_DivTree (moe_routing): per-agent 2-layer MLP.

Math (per batch row b, agent a, with r = routing[a]):
    x0   = concat(x_in[b, a], onehot(a))                  # [H + A]
    h    = relu(x0 @ W1[r] + b1[r])                       # [H]
    out  = h @ W2[r] + b2[r]                              # [NACT]

Host-side simplifications baked in before the device kernel runs:
  - The onehot half of x0 @ W1[r] just selects row H+a of W1[r], so it is
    folded into an effective bias:  bias1e[a] = b1[r] + W1[r, H+a, :].
  - b2[r] is added on the host during unshard (it is constant over batch).
  - Expert weights are gathered by routing on the host (pure indexing).
  - x and W are cast to bf16 on the host: halves HBM traffic and enables
    1-pass FWL weight loads on the PE; measured end-to-end scale-rel error
    ~2.8e-3 (threshold 2e-2). Biases stay fp32 (added in fp32 on ACT).

Sharding: expert-parallel over agents. 48 agents are assigned whole to
cores (6 each); the remaining 2 agents are split into 4 batch-quarters
each (cores 0-3 take agent 48's quarters, cores 4-7 agent 49's), so all
8 cores run an identical program over 25 (agent, batch-512) work units.

Device kernel per unit (bf16 matmuls, fp32 PSUM accumulate):
    hT[m]  = relu(sum_k W1e[k, m-chunk].T @ xT[k-chunk] + bias1e)  # [128, 512] x4
    MM2 is column-tiled on the PE: tile (0,0) handles batch cols 0-255,
    tile (0,64) cols 256-511, each accumulating its 4 k-chunks into its
    own PSUM partition range, so the two M=64 matmul chains run
    concurrently and the 128-wide array is fully used.  The [128, 256]
    PSUM result (nact on partitions 0-63 / 64-127 for the two batch
    halves) is cast to bf16 on DVE and DMA'd out per unit; the host
    untangles the layout during unshard.
"""

import os
import sys

import numpy as np

sys.path.insert(0, "/opt/trn_rl_repo")

B, A, H, NACT = 2048, 50, 512, 64
N_CORES = 8
BT = 512  # batch tile (rows per work unit)
HB = BT // 2  # batch half processed per PE column-tile
FULL_PER_CORE = 6  # whole agents per core
N_UNITS = FULL_PER_CORE * 4 + 1  # 25 work units per core
N_AG = FULL_PER_CORE + 1  # weight slots per core (6 full + 1 split)
KC = H // 128  # 4 contraction chunks
MC = H // 128  # 4 output-hidden chunks
NWARM = 8

LAST_RUN_INFO = {}

_CACHE = {}


def _unit_tables():
    """Per-core unit -> (agent, b0) and weight-slot tables."""
    per_core = []
    for c in range(N_CORES):
        full = list(range(c * FULL_PER_CORE, (c + 1) * FULL_PER_CORE))
        split_agent = 48 + (c // 4)
        quarter = c % 4
        units = [(a, j * BT) for a in full for j in range(4)]
        units.append((split_agent, quarter * BT))
        agents = full + [split_agent]
        per_core.append((units, agents))
    return per_core


def _build_nc():
    import concourse.bacc as bacc
    import concourse.mybir as mybir
    import concourse.tile as tile

    F32 = mybir.dt.float32
    BF16 = mybir.dt.bfloat16
    Relu = mybir.ActivationFunctionType.Relu

    W1C = KC * MC * 128      # 2048 w1 columns
    W2C = KC * NACT          # 256 w2 columns
    SLAB = W1C + W2C         # 2304 bf16 columns

    nc = bacc.Bacc(None)
    xt_d = nc.declare_dram_parameter("xt", [N_UNITS, 128, KC * BT], BF16, isOutput=False)
    ws_d = nc.declare_dram_parameter("ws", [N_AG, 128, SLAB], BF16, isOutput=False)
    bs_d = nc.declare_dram_parameter("bs", [N_AG, 128, MC], F32, isOutput=False)
    out_d = nc.declare_dram_parameter("out", [N_AG, 128, 4 * HB], BF16, isOutput=True)

    with tile.TileContext(nc) as tc:
        with (
            tc.tile_pool(name="xtp", bufs=8) as xtp,
            tc.tile_pool(name="wp", bufs=3) as wp,
            tc.tile_pool(name="bp", bufs=3) as bp,
            tc.tile_pool(name="htp", bufs=10) as htp,
            tc.tile_pool(name="obp", bufs=4) as obp,
            tc.tile_pool(name="ps1p", bufs=6, space="PSUM") as ps1p,
            tc.tile_pool(name="ps2p", bufs=2, space="PSUM") as ps2p,
        ):
            wts = {}
            bts = {}

            def emit_wslab(ai, split_first=False):
                # Agent 0's slab streams k-block-at-a-time on the ACT HWDGE
                # ring (concurrent with xt on Sync) so unit 0 can start on
                # chunk k as soon as it lands.
                wt = wp.tile([128, SLAB], BF16, tag="w", name=f"w_{ai}")
                if split_first:
                    for k in range(KC):
                        nc.scalar.dma_start(
                            out=wt[:, k * MC * 128 : (k + 1) * MC * 128],
                            in_=ws_d[ai][:, k * MC * 128 : (k + 1) * MC * 128],
                        )
                    nc.scalar.dma_start(out=wt[:, W1C:], in_=ws_d[ai][:, W1C:])
                else:
                    nc.scalar.dma_start(out=wt, in_=ws_d[ai])
                bt = bp.tile([128, MC], F32, tag="b", name=f"b_{ai}")
                nc.scalar.dma_start(out=bt, in_=bs_d[ai])
                wts[ai] = wt
                bts[ai] = bt

            def flush(p):
                # MM2 for one finished unit: two concurrent column-tile
                # chains (batch halves), 4 accumulating k-chunks each.
                hts, ai, u, j = p
                wt = wts[ai]
                ps2 = ps2p.tile([128, HB], F32, tag="ps2", name=f"ps2_{u}")
                for k in range(KC):
                    for half in range(2):
                        nc.tensor.matmul(
                            ps2[half * NACT : (half + 1) * NACT, :],
                            lhsT=wt[:, W1C + k * NACT : W1C + (k + 1) * NACT],
                            rhs=hts[k][:, half * HB : (half + 1) * HB],
                            start=(k == 0),
                            stop=(k == KC - 1),
                            tile_position=(0, half * NACT),
                        )
                ob = obp.tile([128, HB], BF16, tag="ob", name=f"ob_{u}")
                nc.vector.tensor_scalar_add(out=ob, in0=ps2, scalar1=0.0)
                nc.scalar.dma_start(
                    out=out_d[ai][:, j * HB : (j + 1) * HB], in_=ob
                )

            emit_wslab(0, split_first=True)

            # Warm the PE (HAM clock gate) with dummy matmuls while the
            # first ws/xt DMAs stream in, so real matmuls start fast.
            warm = htp.tile([128, BT], BF16, tag="warm", name="warm", bufs=1)
            nc.vector.memset(warm, 0.0)
            wps = ps1p.tile([128, BT], F32, tag="ps1", name="warm_ps")
            for r in range(NWARM):
                nc.tensor.matmul(
                    wps[0:NACT, :],
                    lhsT=warm[:, :NACT],
                    rhs=warm,
                    start=(r == 0),
                    stop=(r == NWARM - 1),
                )

            pending = None
            for u in range(N_UNITS):
                ai = u // 4 if u < FULL_PER_CORE * 4 else FULL_PER_CORE
                j = (u % 4) if ai != FULL_PER_CORE else 0

                xt_t = xtp.tile([128, KC * BT], BF16, tag="xt", name=f"xt_{u}")
                if u == 0:
                    for k in range(KC):
                        nc.sync.dma_start(
                            out=xt_t[:, k * BT : (k + 1) * BT],
                            in_=xt_d[u][:, k * BT : (k + 1) * BT],
                        )
                else:
                    nc.sync.dma_start(out=xt_t, in_=xt_d[u])
                if u % 4 == 1 and u // 4 + 1 <= FULL_PER_CORE:
                    emit_wslab(u // 4 + 1)  # one-agent prefetch lead

                wt = wts[ai]
                bt = bts[ai]
                ps1s = [
                    ps1p.tile([128, BT], F32, tag="ps1", name=f"ps1_{u}_{m}")
                    for m in range(MC)
                ]
                # Unit 0 runs k-outer so each arriving k-chunk of xt/w1
                # unlocks 4 matmuls; later units run m-outer so each psum
                # tile completes early and its relu overlaps the next m.
                order = (
                    [(m, k) for k in range(KC) for m in range(MC)]
                    if u == 0
                    else [(m, k) for m in range(MC) for k in range(KC)]
                )
                hts = [None] * MC
                for m, k in order:
                    nc.tensor.matmul(
                        ps1s[m],
                        lhsT=wt[:, (k * MC + m) * 128 : (k * MC + m + 1) * 128],
                        rhs=xt_t[:, k * BT : (k + 1) * BT],
                        start=(k == 0),
                        stop=(k == KC - 1),
                    )
                    if k == KC - 1:
                        ht = htp.tile([128, BT], BF16, tag="ht", name=f"ht_{u}_{m}")
                        if m % 2 == 0:
                            nc.scalar.activation(
                                out=ht,
                                in_=ps1s[m],
                                func=Relu,
                                bias=bt[:, m : m + 1],
                            )
                        else:
                            # relu+bias on DVE: out = max(in + bias, 0).
                            # Splitting relus across ACT and DVE halves the
                            # per-unit relu serial chain (tail latency).
                            nc.vector.tensor_scalar(
                                out=ht,
                                in0=ps1s[m],
                                scalar1=bt[:, m : m + 1],
                                scalar2=0.0,
                                op0=mybir.AluOpType.add,
                                op1=mybir.AluOpType.max,
                            )
                        hts[m] = ht

                if pending is not None:
                    flush(pending)
                pending = (hts, ai, u, j)
            flush(pending)

    nc.finalize()
    return nc


def _prep_inputs(x_in, W1, b1, W2, b2, routing):
    """Host-side: routing gather, onehot fold, bf16 cast, per-core tiling."""
    import ml_dtypes

    BF = ml_dtypes.bfloat16
    x_in = np.ascontiguousarray(x_in, dtype=np.float32)
    W1 = np.asarray(W1, dtype=np.float32)
    b1 = np.asarray(b1, dtype=np.float32)
    W2 = np.asarray(W2, dtype=np.float32)
    b2 = np.asarray(b2, dtype=np.float32)
    routing = np.asarray(routing)

    W1C = KC * MC * 128
    W2C = KC * NACT
    SLAB = W1C + W2C

    W1r = W1[routing]  # [A, H+A, H]
    W2r = W2[routing]  # [A, H, NACT]
    bias1e = b1[routing] + W1r[np.arange(A), H + np.arange(A), :]  # [A, H]
    b2e = b2[routing]  # [A, NACT]

    ws_all = np.empty((A, 128, SLAB), dtype=BF)
    # w1 block: [p, (k*MC+m)*128+c] = W1r[a, k*128+p, m*128+c]
    ws_all[:, :, :W1C] = (
        W1r[:, :H, :]
        .reshape(A, KC, 128, MC, 128)
        .transpose(0, 2, 1, 3, 4)
        .reshape(A, 128, W1C)
        .astype(BF)
    )
    # w2 block: [p, k*NACT+n] = W2r[a, k*128+p, n]
    ws_all[:, :, W1C:] = (
        W2r.reshape(A, KC, 128, NACT).transpose(0, 2, 1, 3).reshape(A, 128, W2C)
        .astype(BF)
    )
    # bias1 cols (fp32, separate tensor)
    bs_all = np.ascontiguousarray(
        bias1e.reshape(A, MC, 128).transpose(0, 2, 1), dtype=np.float32
    )

    x16 = x_in.astype(BF)  # one pass over the full input
    per_core = _unit_tables()
    in_maps = []
    for c in range(N_CORES):
        units, agents = per_core[c]
        xt = np.empty((N_UNITS, 128, KC * BT), dtype=BF)
        for u, (a, b0) in enumerate(units):
            # [p, k*BT+b] = x_in[b0+b, a, k*128+p]
            xs = x16[b0 : b0 + BT, a, :]  # [BT, H]
            xt[u] = xs.T.reshape(KC, 128, BT).transpose(1, 0, 2).reshape(128, KC * BT)
        in_maps.append(
            {
                "xt": xt,
                "ws": np.ascontiguousarray(ws_all[agents]),
                "bs": np.ascontiguousarray(bs_all[agents]),
            }
        )
    return in_maps, per_core, b2e


def _install_ntff_hook():
    import types

    try:
        from antenv.axon_hooks import get_axon_ntff_profile_hook  # noqa: F401

        return
    except ImportError:
        pass
    try:
        import antenv
        from trn_agent_boot.trn_boot import _ntff_profile_via_ctypes

        hook = _ntff_profile_via_ctypes("/opt/axon/libaxon_pjrt.so")
        mod = types.ModuleType("antenv.axon_hooks")
        mod.get_axon_ntff_profile_hook = lambda: hook
        mod.set_axon_ntff_profile_hook = lambda h: None
        sys.modules["antenv.axon_hooks"] = mod
        antenv.axon_hooks = mod
    except Exception:
        pass


def kernel(x_in, W1, b1, W2, b2, routing):
    from concourse.bass_utils import run_bass_kernel_spmd

    trace = bool(os.environ.get("TRN_KERNEL_TRACE"))
    if trace:
        _install_ntff_hook()

    if "nc" not in _CACHE:
        _CACHE["nc"] = _build_nc()
    nc = _CACHE["nc"]

    in_maps, per_core, b2e = _prep_inputs(x_in, W1, b1, W2, b2, routing)

    kwargs = {}
    if trace:
        kwargs = dict(trace=True, tmpdir=os.environ.get("TRN_KERNEL_TRACE_DIR") or None)
    res = run_bass_kernel_spmd(nc, in_maps, core_ids=list(range(N_CORES)), **kwargs)

    LAST_RUN_INFO.clear()
    LAST_RUN_INFO["exec_time_ns"] = res.exec_time_ns
    LAST_RUN_INFO["results"] = res

    out_full = np.empty((B, A, NACT), dtype=np.float32)
    for c in range(N_CORES):
        units, agents = per_core[c]
        oc = np.asarray(res.results[c]["out"]).astype(np.float32)  # [N_AG,128,4*HB]
        for ai, a in enumerate(agents):
            if ai == FULL_PER_CORE:
                b0 = units[-1][1]
                blk = oc[ai, :, :HB]
                out_full[b0 : b0 + HB, a, :] = blk[:NACT].T
                out_full[b0 + HB : b0 + BT, a, :] = blk[NACT:].T
            else:
                for j in range(4):
                    b0 = j * BT
                    blk = oc[ai, :, j * HB : (j + 1) * HB]
                    out_full[b0 : b0 + HB, a, :] = blk[:NACT].T
                    out_full[b0 + HB : b0 + BT, a, :] = blk[NACT:].T
    out_full += b2e[None, :, :]
    return out_full


# revision 13
# speedup vs baseline: 1.1840x; 1.1840x over previous
"""TRN2 Bass kernel for nn_DivTree (moe_routing): per-agent 2-layer MLP.

Math (per batch row b, agent a, with r = routing[a]):
    x0   = concat(x_in[b, a], onehot(a))                  # [H + A]
    h    = relu(x0 @ W1[r] + b1[r])                       # [H]
    out  = h @ W2[r] + b2[r]                              # [NACT]

Host-side simplifications baked in before the device kernel runs:
  - The onehot half of x0 @ W1[r] just selects row H+a of W1[r], so it is
    folded into an effective bias:  bias1e[a] = b1[r] + W1[r, H+a, :].
  - b2[r] is added on the host during unshard (it is constant over batch).
  - Expert weights are gathered by routing on the host (pure indexing).
  - x and W are cast to bf16 on the host: halves HBM traffic and enables
    1-pass FWL weight loads on the PE; measured end-to-end scale-rel error
    ~2.8e-3 (threshold 2e-2). Biases stay fp32 (added in fp32 on ACT).

Sharding: expert-parallel over agents. 48 agents are assigned whole to
cores (6 each); the remaining 2 agents are split into 4 batch-quarters
each (cores 0-3 take agent 48's quarters, cores 4-7 agent 49's), so all
8 cores run an identical program over 25 (agent, batch-512) work units.

Device kernel per unit (bf16 matmuls, fp32 PSUM accumulate):
    hT[m]  = relu(sum_k W1e[k, m-chunk].T @ xT[k-chunk] + bias1e)  # [128, 512] x4
    MM2 is column-tiled on the PE: tile (0,0) handles batch cols 0-255,
    tile (0,64) cols 256-511, each accumulating its 4 k-chunks into its
    own PSUM partition range, so the two M=64 matmul chains run
    concurrently and the 128-wide array is fully used.  The [128, 256]
    PSUM result (nact on partitions 0-63 / 64-127 for the two batch
    halves) is cast to bf16 on DVE and DMA'd out per unit; the host
    untangles the layout during unshard.
"""

import os
import sys

import numpy as np

sys.path.insert(0, "/opt/trn_rl_repo")

B, A, H, NACT = 2048, 50, 512, 64
N_CORES = 8
BT = 512  # batch tile (rows per work unit)
HB = BT // 2  # batch half processed per PE column-tile
FULL_PER_CORE = 6  # whole agents per core
N_UNITS = FULL_PER_CORE * 4 + 1  # 25 work units per core
N_AG = FULL_PER_CORE + 1  # weight slots per core (6 full + 1 split)
KC = H // 128  # 4 contraction chunks
MC = H // 128  # 4 output-hidden chunks
NWARM = 8

LAST_RUN_INFO = {}

_CACHE = {}


def _unit_tables():
    """Per-core unit -> (agent, b0) and weight-slot tables."""
    per_core = []
    for c in range(N_CORES):
        full = list(range(c * FULL_PER_CORE, (c + 1) * FULL_PER_CORE))
        split_agent = 48 + (c // 4)
        quarter = c % 4
        units = [(a, j * BT) for a in full for j in range(4)]
        units.append((split_agent, quarter * BT))
        agents = full + [split_agent]
        per_core.append((units, agents))
    return per_core


def _build_nc():
    import concourse.bacc as bacc
    import concourse.mybir as mybir
    import concourse.tile as tile

    F32 = mybir.dt.float32
    BF16 = mybir.dt.bfloat16
    Relu = mybir.ActivationFunctionType.Relu

    W1C = KC * MC * 128      # 2048 w1 columns
    W2C = KC * NACT          # 256 w2 columns
    SLAB = W1C + W2C         # 2304 bf16 columns

    nc = bacc.Bacc(None)
    xt_d = nc.declare_dram_parameter("xt", [N_UNITS, 128, KC * BT], BF16, isOutput=False)
    ws_d = nc.declare_dram_parameter("ws", [N_AG, 128, SLAB], BF16, isOutput=False)
    bs_d = nc.declare_dram_parameter("bs", [N_AG, 128, MC], F32, isOutput=False)
    out_d = nc.declare_dram_parameter("out", [N_AG, 128, 4 * HB], BF16, isOutput=True)

    with tile.TileContext(nc) as tc:
        with (
            tc.tile_pool(name="xtp", bufs=8) as xtp,
            tc.tile_pool(name="wp", bufs=3) as wp,
            tc.tile_pool(name="bp", bufs=3) as bp,
            tc.tile_pool(name="htp", bufs=16) as htp,
            tc.tile_pool(name="obp", bufs=6) as obp,
            tc.tile_pool(name="ps1p", bufs=6, space="PSUM") as ps1p,
            tc.tile_pool(name="ps2p", bufs=2, space="PSUM") as ps2p,
        ):
            wts = {}
            bts = {}

            def emit_wslab(ai, split_first=False):
                # Agent 0's slab streams k-block-at-a-time on the ACT HWDGE
                # ring (concurrent with xt on Sync) so unit 0 can start on
                # chunk k as soon as it lands.
                wt = wp.tile([128, SLAB], BF16, tag="w", name=f"w_{ai}")
                if split_first:
                    for k in range(KC):
                        nc.scalar.dma_start(
                            out=wt[:, k * MC * 128 : (k + 1) * MC * 128],
                            in_=ws_d[ai][:, k * MC * 128 : (k + 1) * MC * 128],
                        )
                    nc.scalar.dma_start(out=wt[:, W1C:], in_=ws_d[ai][:, W1C:])
                else:
                    nc.scalar.dma_start(out=wt, in_=ws_d[ai])
                bt = bp.tile([128, MC], F32, tag="b", name=f"b_{ai}")
                nc.scalar.dma_start(out=bt, in_=bs_d[ai])
                wts[ai] = wt
                bts[ai] = bt

            def flush(p):
                # MM2 for one finished unit: two concurrent column-tile
                # chains (batch halves), 4 accumulating k-chunks each.
                hts, ai, u, j = p
                wt = wts[ai]
                ps2 = ps2p.tile([128, HB], F32, tag="ps2", name=f"ps2_{u}")
                for k in range(KC):
                    for half in range(2):
                        nc.tensor.matmul(
                            ps2[half * NACT : (half + 1) * NACT, :],
                            lhsT=wt[:, W1C + k * NACT : W1C + (k + 1) * NACT],
                            rhs=hts[k][:, half * HB : (half + 1) * HB],
                            start=(k == 0),
                            stop=(k == KC - 1),
                            tile_position=(0, half * NACT),
                        )
                ob = obp.tile([128, HB], BF16, tag="ob", name=f"ob_{u}")
                nc.vector.tensor_scalar_add(out=ob, in0=ps2, scalar1=0.0)
                nc.gpsimd.dma_start(
                    out=out_d[ai][:, j * HB : (j + 1) * HB], in_=ob
                )

            emit_wslab(0, split_first=True)

            # Warm the PE (HAM clock gate) with dummy matmuls while the
            # first ws/xt DMAs stream in, so real matmuls start fast.
            warm = htp.tile([128, BT], BF16, tag="warm", name="warm", bufs=1)
            nc.vector.memset(warm, 0.0)
            wps = ps1p.tile([128, BT], F32, tag="ps1", name="warm_ps")
            for r in range(NWARM):
                nc.tensor.matmul(
                    wps[0:NACT, :],
                    lhsT=warm[:, :NACT],
                    rhs=warm,
                    start=(r == 0),
                    stop=(r == NWARM - 1),
                )

            pending = []
            for u in range(N_UNITS):
                ai = u // 4 if u < FULL_PER_CORE * 4 else FULL_PER_CORE
                j = (u % 4) if ai != FULL_PER_CORE else 0

                xt_t = xtp.tile([128, KC * BT], BF16, tag="xt", name=f"xt_{u}")
                if u == 0:
                    for k in range(KC):
                        nc.sync.dma_start(
                            out=xt_t[:, k * BT : (k + 1) * BT],
                            in_=xt_d[u][:, k * BT : (k + 1) * BT],
                        )
                else:
                    nc.sync.dma_start(out=xt_t, in_=xt_d[u])
                if u % 4 == 1 and u // 4 + 1 <= FULL_PER_CORE:
                    emit_wslab(u // 4 + 1)  # one-agent prefetch lead

                wt = wts[ai]
                bt = bts[ai]
                ps1s = [
                    ps1p.tile([128, BT], F32, tag="ps1", name=f"ps1_{u}_{m}")
                    for m in range(MC)
                ]
                # Unit 0 runs k-outer so each arriving k-chunk of xt/w1
                # unlocks 4 matmuls; later units run m-outer so each psum
                # tile completes early and its relu overlaps the next m.
                order = (
                    [(m, k) for k in range(KC) for m in range(MC)]
                    if u == 0
                    else [(m, k) for m in range(MC) for k in range(KC)]
                )
                hts = [None] * MC
                for m, k in order:
                    nc.tensor.matmul(
                        ps1s[m],
                        lhsT=wt[:, (k * MC + m) * 128 : (k * MC + m + 1) * 128],
                        rhs=xt_t[:, k * BT : (k + 1) * BT],
                        start=(k == 0),
                        stop=(k == KC - 1),
                    )
                    if k == KC - 1:
                        ht = htp.tile([128, BT], BF16, tag="ht", name=f"ht_{u}_{m}")
                        nc.scalar.activation(
                            out=ht,
                            in_=ps1s[m],
                            func=Relu,
                            bias=bt[:, m : m + 1],
                        )
                        hts[m] = ht

                # Flush two finished units back-to-back every other unit
                # (pair {u-2, u-1} lands after MM1 of unit u): halves the
                # number of full-width <-> column-tiled PE reconfiguration
                # boundaries (each costs a pipeline drain), while keeping
                # at least one unit of deferral so the relus are done.
                pending.append((hts, ai, u, j))
                if u % 2 == 0 and len(pending) >= 3:
                    flush(pending.pop(0))
                    flush(pending.pop(0))
            for p in pending:
                flush(p)

    nc.finalize()
    return nc


def _prep_inputs(x_in, W1, b1, W2, b2, routing):
    """Host-side: routing gather, onehot fold, bf16 cast, per-core tiling."""
    import ml_dtypes

    BF = ml_dtypes.bfloat16
    x_in = np.ascontiguousarray(x_in, dtype=np.float32)
    W1 = np.asarray(W1, dtype=np.float32)
    b1 = np.asarray(b1, dtype=np.float32)
    W2 = np.asarray(W2, dtype=np.float32)
    b2 = np.asarray(b2, dtype=np.float32)
    routing = np.asarray(routing)

    W1C = KC * MC * 128
    W2C = KC * NACT
    SLAB = W1C + W2C

    W1r = W1[routing]  # [A, H+A, H]
    W2r = W2[routing]  # [A, H, NACT]
    bias1e = b1[routing] + W1r[np.arange(A), H + np.arange(A), :]  # [A, H]
    b2e = b2[routing]  # [A, NACT]

    ws_all = np.empty((A, 128, SLAB), dtype=BF)
    # w1 block: [p, (k*MC+m)*128+c] = W1r[a, k*128+p, m*128+c]
    ws_all[:, :, :W1C] = (
        W1r[:, :H, :]
        .reshape(A, KC, 128, MC, 128)
        .transpose(0, 2, 1, 3, 4)
        .reshape(A, 128, W1C)
        .astype(BF)
    )
    # w2 block: [p, k*NACT+n] = W2r[a, k*128+p, n]
    ws_all[:, :, W1C:] = (
        W2r.reshape(A, KC, 128, NACT).transpose(0, 2, 1, 3).reshape(A, 128, W2C)
        .astype(BF)
    )
    # bias1 cols (fp32, separate tensor)
    bs_all = np.ascontiguousarray(
        bias1e.reshape(A, MC, 128).transpose(0, 2, 1), dtype=np.float32
    )

    x16 = x_in.astype(BF)  # one pass over the full input
    per_core = _unit_tables()
    in_maps = []
    for c in range(N_CORES):
        units, agents = per_core[c]
        xt = np.empty((N_UNITS, 128, KC * BT), dtype=BF)
        for u, (a, b0) in enumerate(units):
            # [p, k*BT+b] = x_in[b0+b, a, k*128+p]
            xs = x16[b0 : b0 + BT, a, :]  # [BT, H]
            xt[u] = xs.T.reshape(KC, 128, BT).transpose(1, 0, 2).reshape(128, KC * BT)
        in_maps.append(
            {
                "xt": xt,
                "ws": np.ascontiguousarray(ws_all[agents]),
                "bs": np.ascontiguousarray(bs_all[agents]),
            }
        )
    return in_maps, per_core, b2e


def _install_ntff_hook():
    import types

    try:
        from antenv.axon_hooks import get_axon_ntff_profile_hook  # noqa: F401

        return
    except ImportError:
        pass
    try:
        import antenv
        from trn_agent_boot.trn_boot import _ntff_profile_via_ctypes

        hook = _ntff_profile_via_ctypes("/opt/axon/libaxon_pjrt.so")
        mod = types.ModuleType("antenv.axon_hooks")
        mod.get_axon_ntff_profile_hook = lambda: hook
        mod.set_axon_ntff_profile_hook = lambda h: None
        sys.modules["antenv.axon_hooks"] = mod
        antenv.axon_hooks = mod
    except Exception:
        pass


def kernel(x_in, W1, b1, W2, b2, routing):
    from concourse.bass_utils import run_bass_kernel_spmd

    trace = bool(os.environ.get("TRN_KERNEL_TRACE"))
    if trace:
        _install_ntff_hook()

    if "nc" not in _CACHE:
        _CACHE["nc"] = _build_nc()
    nc = _CACHE["nc"]

    in_maps, per_core, b2e = _prep_inputs(x_in, W1, b1, W2, b2, routing)

    kwargs = {}
    if trace:
        kwargs = dict(trace=True, tmpdir=os.environ.get("TRN_KERNEL_TRACE_DIR") or None)
    res = run_bass_kernel_spmd(nc, in_maps, core_ids=list(range(N_CORES)), **kwargs)

    LAST_RUN_INFO.clear()
    LAST_RUN_INFO["exec_time_ns"] = res.exec_time_ns
    LAST_RUN_INFO["results"] = res

    out_full = np.empty((B, A, NACT), dtype=np.float32)
    for c in range(N_CORES):
        units, agents = per_core[c]
        oc = np.asarray(res.results[c]["out"]).astype(np.float32)  # [N_AG,128,4*HB]
        for ai, a in enumerate(agents):
            if ai == FULL_PER_CORE:
                b0 = units[-1][1]
                blk = oc[ai, :, :HB]
                out_full[b0 : b0 + HB, a, :] = blk[:NACT].T
                out_full[b0 + HB : b0 + BT, a, :] = blk[NACT:].T
            else:
                for j in range(4):
                    b0 = j * BT
                    blk = oc[ai, :, j * HB : (j + 1) * HB]
                    out_full[b0 : b0 + HB, a, :] = blk[:NACT].T
                    out_full[b0 + HB : b0 + BT, a, :] = blk[NACT:].T
    out_full += b2e[None, :, :]
    return out_full
